# revision 11
# baseline (speedup 1.0000x reference)
"""Causal single-head attention forward (B=4, T=4096, C=256, H=64) on 8 NeuronCores.

Sharding: core = (batch, kv_parity).  Each core processes ALL queries of its
batch, but only the kv tiles (128 keys each) whose global tile index has its
parity (even/odd interleave) -- this balances the causal workload exactly
across the two cores of a batch.  Each core emits the *unnormalized* softmax
numerator (exp(S) @ V, transposed: [H, T]) plus the denominator row [1, T];
the host merges the two partials per batch: out = (u0+u1)/(d0+d1), then
transposes back.

On-chip layout is "transposed activation" space: host passes xT = x[b].T so
projections contract C on partitions; scores are computed transposed
(S^T = K Q^T, [keys, q]) so the AV matmul can use V in natural [keys, H]
layout as the stationary operand with keys as the contraction dim.  A column
of ones appended to V folds the softmax denominator into the same matmul.
"""

import sys

for _p in ("/opt/trn_rl_repo", "/root/.axon_site/_ro/trn_rl_repo"):
    if _p not in sys.path:
        sys.path.append(_p)

from contextlib import ExitStack

import numpy as np

import concourse.bacc as bacc
import concourse.bass as bass
import concourse.tile as tile
from concourse import mybir
from concourse.bass_utils import run_bass_kernel_spmd

B, T, C, H = 4, 4096, 256, 64
TK = T // 2      # kv columns owned by one core (its parity's tiles)
QB = 512         # query block width
NQB = T // QB    # 8 query blocks
KT = 128         # kv tile width
F32 = mybir.dt.float32
BF16 = mybir.dt.bfloat16
SCALE = float(C) ** -0.5

_NC = None


def build_nc() -> bass.Bass:
    nc = bacc.Bacc("TRN2", target_bir_lowering=False, debug=False)
    xT = nc.declare_dram_parameter("xT", [C, T], BF16, isOutput=False)
    xkvT = nc.declare_dram_parameter("xkvT", [C, TK], BF16, isOutput=False)
    wq = nc.declare_dram_parameter("wq", [C, H], BF16, isOutput=False)
    wk = nc.declare_dram_parameter("wk", [C, H], BF16, isOutput=False)
    wv = nc.declare_dram_parameter("wv", [C, H], BF16, isOutput=False)
    masks = nc.declare_dram_parameter("masks", [KT, 2 * QB], BF16, isOutput=False)
    uT = nc.declare_dram_parameter("uT", [H, T], F32, isOutput=True)
    den = nc.declare_dram_parameter("den", [1, T], F32, isOutput=True)

    with tile.TileContext(nc) as tc, ExitStack() as ctx:
        persist = ctx.enter_context(tc.tile_pool(name="persist", bufs=1))
        pexp = ctx.enter_context(tc.tile_pool(name="exp", bufs=4))
        pout = ctx.enter_context(tc.tile_pool(name="out", bufs=2))
        pproj = ctx.enter_context(tc.tile_pool(name="pproj", bufs=1, space="PSUM"))
        pqk = ctx.enter_context(tc.tile_pool(name="pqk", bufs=3, space="PSUM"))
        pav = ctx.enter_context(tc.tile_pool(name="pav", bufs=1, space="PSUM"))

        # ---- load weights + masks ------------------------------------------
        w_sb = {}
        for name, dram in (("q", wq), ("k", wk), ("v", wv)):
            for cc in range(2):
                t = persist.tile([128, H], BF16, tag=f"w{name}{cc}")
                nc.sync.dma_start(out=t[:], in_=dram[128 * cc : 128 * (cc + 1), :])
                w_sb[name, cc] = t
        m_sb = persist.tile([KT, 2 * QB], BF16, tag="mask")
        nc.sync.dma_start(out=m_sb[:], in_=masks[:])

        # ---- load xT (full, for Q) and xkvT (gathered, for K/V) ------------
        # j-major emission so early q-blocks' data (and kv data) arrive first
        x_sb = {}
        xkv_sb = {}
        for j in range(NQB):
            for cc in range(2):
                t = persist.tile([128, QB], BF16, tag=f"x{cc}_{j}")
                nc.sync.dma_start(
                    out=t[:], in_=xT[128 * cc : 128 * (cc + 1), QB * j : QB * (j + 1)]
                )
                x_sb[cc, j] = t
            if j < TK // QB:
                for cc in range(2):
                    t = persist.tile([128, QB], BF16, tag=f"xk{cc}_{j}")
                    nc.sync.dma_start(
                        out=t[:],
                        in_=xkvT[128 * cc : 128 * (cc + 1), QB * j : QB * (j + 1)],
                    )
                    xkv_sb[cc, j] = t

        # ---- projections ----------------------------------------------------
        # qT[64, T] in 8 blocks; kT[64, TK] in 4 blocks; contract C in 2 chunks
        q_sb = [None] * NQB
        k_sb = [None] * (TK // QB)
        v_sb = [None] * (TK // KT)

        def proj_q(j):
            ps = pproj.tile([64, QB], F32, tag="proj")
            for cc in range(2):
                nc.tensor.matmul(
                    ps[:], lhsT=w_sb["q", cc][:], rhs=x_sb[cc, j][:],
                    start=(cc == 0), stop=(cc == 1),
                )
            t = persist.tile([64, QB], BF16, tag=f"qT{j}")
            nc.vector.tensor_copy(t[:], ps[:])
            q_sb[j] = t

        def proj_k(j):
            ps = pproj.tile([64, QB], F32, tag="proj")
            for cc in range(2):
                nc.tensor.matmul(
                    ps[:], lhsT=w_sb["k", cc][:], rhs=xkv_sb[cc, j][:],
                    start=(cc == 0), stop=(cc == 1),
                )
            t = persist.tile([64, QB], BF16, tag=f"kT{j}")
            nc.vector.tensor_copy(t[:], ps[:])
            k_sb[j] = t

        def proj_v(tt):
            # v natural [keys, H] + ones column (denominator fold):
            # lhsT = xkvT chunk (stationary), rhs = Wv chunk
            ps = pproj.tile([128, H], F32, tag="proj")
            j, o = divmod(tt, 4)
            for cc in range(2):
                nc.tensor.matmul(
                    ps[:],
                    lhsT=xkv_sb[cc, j][:, KT * o : KT * (o + 1)],
                    rhs=w_sb["v", cc][:],
                    start=(cc == 0), stop=(cc == 1),
                )
            t = persist.tile([128, H + 1], BF16, tag=f"v{tt}")
            nc.vector.tensor_copy(t[:, 0:H], ps[:])
            nc.vector.memset(t[:, H : H + 1], 1.0)
            v_sb[tt] = t

        # interleave so q-block 0's dependencies are produced first
        for j in range(TK // QB):
            proj_k(j)
            for tt in range(4 * j, 4 * j + 4):
                proj_v(tt)
            proj_q(2 * j)
            proj_q(2 * j + 1)

        # ---- attention ------------------------------------------------------
        # kv tiles in pairs: one [128, 2*QB] psum (2 banks) per pair, a single
        # exp over both; AV matmuls run one pair BEHIND the QK/exp stream so
        # the in-order PE never blocks on ACT's current exp.
        for p in range(NQB):
            ns = 2 * (p + 1)  # local kv tiles visible to this q block
            npair = ns // 2
            av = pav.tile([H + 1, QB], F32, tag="av")

            def av_pair(P, exP, last):
                for half in range(2):
                    s = 2 * P + half
                    nc.tensor.matmul(
                        av[:], lhsT=v_sb[s][:],
                        rhs=exP[:, QB * half : QB * (half + 1)],
                        start=(s == 0), stop=(last and half == 1),
                    )

            exs = []
            for P in range(npair):
                qk2 = pqk.tile([KT, 2 * QB], F32, tag="qk")
                for half in range(2):
                    s = 2 * P + half
                    j, o = divmod(s, 4)
                    nc.tensor.matmul(
                        qk2[:, QB * half : QB * (half + 1)],
                        lhsT=k_sb[j][:, KT * o : KT * (o + 1)],
                        rhs=q_sb[p][:],
                        start=True, stop=True,
                    )
                ex = pexp.tile([KT, 2 * QB], BF16, tag="exp")
                nc.scalar.activation(
                    ex[:], qk2[:], mybir.ActivationFunctionType.Exp, scale=SCALE
                )
                if P == npair - 1:
                    nc.vector.tensor_mul(ex[:], ex[:], m_sb[:])
                exs.append(ex)
                if P >= 1:
                    av_pair(P - 1, exs[P - 1], last=False)
            av_pair(npair - 1, exs[npair - 1], last=True)

            ot = pout.tile([H + 1, QB], F32, tag="out")
            nc.vector.tensor_copy(ot[:], av[:])
            nc.sync.dma_start(out=uT[:, QB * p : QB * (p + 1)], in_=ot[0:H, :])
            nc.sync.dma_start(out=den[:, QB * p : QB * (p + 1)], in_=ot[H : H + 1, :])

    nc.compile()
    return nc


def get_nc() -> bass.Bass:
    global _NC
    if _NC is None:
        _NC = build_nc()
    return _NC


def make_in_maps(x, Wk, Wq, Wv):
    import ml_dtypes

    bf16 = ml_dtypes.bfloat16
    x = np.ascontiguousarray(np.asarray(x, np.float32).astype(bf16))
    Wk = np.ascontiguousarray(np.asarray(Wk, np.float32).astype(bf16))
    Wq = np.ascontiguousarray(np.asarray(Wq, np.float32).astype(bf16))
    Wv = np.ascontiguousarray(np.asarray(Wv, np.float32).astype(bf16))
    jj = np.arange(QB)[None, :]
    kk = np.arange(KT)[:, None]
    M = [(jj >= kk + KT * d).astype(bf16) for d in range(4)]
    in_maps = []
    for core in range(8):
        b, par = divmod(core, 2)
        xTb = np.ascontiguousarray(x[b].T)  # [C, T]
        # gather this parity's kv tiles: global tile g = 2s+par -> local slot s
        cols = (
            (2 * np.arange(TK // KT)[:, None] + par) * KT + np.arange(KT)[None, :]
        ).reshape(-1)
        xkvT = np.ascontiguousarray(xTb[:, cols])
        in_maps.append(
            {
                "xT": xTb,
                "xkvT": xkvT,
                "wq": Wq,
                "wk": Wk,
                "wv": Wv,
                "masks": np.ascontiguousarray(
                    np.concatenate([M[par], M[par + 2]], axis=1)
                ),
            }
        )
    return in_maps


def merge(results):
    out = np.empty((B, T, H), np.float32)
    for b in range(B):
        num = results[2 * b]["uT"] + results[2 * b + 1]["uT"]  # [H, T]
        d = results[2 * b]["den"] + results[2 * b + 1]["den"]  # [1, T]
        out[b] = (num / d).T
    return out


def kernel(x, Wk, Wq, Wv, **kw):
    in_maps = make_in_maps(x, Wk, Wq, Wv)
    res = run_bass_kernel_spmd(get_nc(), in_maps, core_ids=list(range(8)), **kw)
    out = merge(res.results)
    if kw:
        return out, res
    return out
